# revision 27
# baseline (speedup 1.0000x reference)
"""Trainium2 Bass kernel for nn_Decoder (LSTM decoder over encoder features).

Math (per reference):
    feats = enc @ W_proj + b_proj            [B, T, DF]
    word  = embed[start_ids]                 [B, DW]   (constant per step)
    x_t   = concat(feats_t, word)
    gates = x_t @ W_ih.T + h @ W_hh.T + b    -> LSTM cell -> h_t (output)

Kernel strategy (8 cores, data-parallel over batch, B_local = 64):
  Everything on-device is kept "gate-major" (transposed: [dims, batch]) so no
  on-device transposes are ever needed:
    GEMM1: feats.T[DF, (t,b)] = W_proj(lhsT) @ enc.T(rhs)   streamed, per-chunk
    GEMM2: XG.T[4H, (t,b)]    = W_x.T(lhsT) @ feats.T(rhs)
           (word/proj bias folded in during the PSUM->SBUF evacuation)
    rec_t: gates.T[4H, b] = XG.T (identity-matmul inject, clears the bank)
                          + W_hh.T(lhsT) @ h.T(rhs)  accumulated in PSUM
    eltwise: sigmoid/tanh on ACT straight from PSUM, muls/adds split across
             DVE and GpSimd; c kept fp32; h written once as bf16 directly
             into the output staging tile (state == output).
  The word-embedding gather, the W_proj/W_ih split, the gate-row permutation
  and all layout transposes are host-side precomputation (numpy only).

  Gate rows are permuted to [f0 i0 o0 g0 f1 i1 o1 g1] (256-row blocks) so the
  ACT evacuation needs few ops and sigmoid(f) lands first (shortens the
  serial recurrence chain).  Gates are split into two PSUM banks (halves) so
  the two halves' eltwise chains interleave.  W_hh is stored fp8-e3m4
  pre-scaled x128 (faster LDWEIGHTS + half the DMA); h is stored as h/128 in
  bf16 (exact power-of-2), so the PSUM product is unscaled; the host
  multiplies the bf16 output back by 128.
"""

import numpy as np
import ml_dtypes

BF16 = ml_dtypes.bfloat16
F8E3 = ml_dtypes.float8_e3m4

# Problem dims (hardcoded per spec)
NCORES = 8
B, T, DE, DF, DW, H, V = 512, 80, 1024, 512, 512, 512, 10000
G4 = 4 * H                      # 2048 gate rows
BL = B // NCORES                # 64 batch per core
CH = 4                          # timesteps per GEMM chunk
NCH = T // CH                   # 20 chunks
KDE = DE // 128                 # 8  contraction chunks for GEMM1
KDF = DF // 128                 # 4  contraction chunks for GEMM2
KH = H // 128                   # 4  contraction chunks for recurrence
MT = G4 // 128                  # 16 gate-row tiles
NCOL = CH * BL                  # 256 (t,b) columns per chunk
WHH_SCALE = 128.0               # W_hh stored x128 in fp8, h stored /128

_COMPILED = None


def _build():
    import concourse.bacc as bacc
    import concourse.tile as tile
    import concourse.mybir as mybir
    import concourse.bass as bass

    dt = mybir.dt
    f32, b16, f8 = dt.float32, dt.bfloat16, dt.float8e3
    AF = mybir.ActivationFunctionType
    ALU = mybir.AluOpType

    nc = bacc.Bacc("TRN2", target_bir_lowering=False, debug=False,
                   num_devices=NCORES)

    ident_d = nc.dram_tensor("ident", [128, 128], b16, kind="ExternalInput")
    encT_d = nc.dram_tensor("encT", [128, NCH, KDE * NCOL], b16, kind="ExternalInput")
    wproj_d = nc.dram_tensor("wproj", [128, KDE, DF], b16, kind="ExternalInput")
    wx_d = nc.dram_tensor("wx", [128, KDF, G4], b16, kind="ExternalInput")
    whh_d = nc.dram_tensor("whh", [128, KH, G4], f8, kind="ExternalInput")
    biasr_d = nc.dram_tensor("biasr", [128, MT, BL], b16, kind="ExternalInput")
    hT_d = nc.dram_tensor("hT", [128, KH, T, BL], b16, kind="ExternalOutput")

    with tile.TileContext(nc) as tc:
        with (
            tc.tile_pool(name="wpool", bufs=1) as wp,
            tc.tile_pool(name="encp", bufs=3) as encp,
            tc.tile_pool(name="featsp", bufs=3) as fp,
            tc.tile_pool(name="xgp", bufs=2) as xgp,
            tc.tile_pool(name="statep", bufs=1) as stp,
            tc.tile_pool(name="ewp", bufs=14) as ewp,
            tc.tile_pool(name="houtpA", bufs=3) as hopA,
            tc.tile_pool(name="houtpB", bufs=3) as hopB,
            tc.tile_pool(name="psg", bufs=2, space=bass.MemorySpace.PSUM) as psg,
            tc.tile_pool(name="ps1", bufs=1, space=bass.MemorySpace.PSUM) as ps1,
            tc.tile_pool(name="ps2", bufs=3, space=bass.MemorySpace.PSUM) as ps2,
        ):
            # ---- persistent weights ----
            wproj_sb = wp.tile([128, KDE * DF], b16)
            wx_q = [wp.tile([128, KDF * 512], b16, name=f"wxq{q}") for q in range(4)]
            whh_k = [wp.tile([128, G4], f8, name=f"whhk{k}") for k in range(KH)]
            biasr_sb = wp.tile([128, MT * BL], b16, name="biasr")
            ident_sb = wp.tile([128, 128], b16)
            nc.sync.dma_start(ident_sb[:], ident_d[:])
            nc.sync.dma_start(wproj_sb[:], wproj_d[:])

            # warmup: keep PE busy (and HAM un-throttled) while DMAs land
            warm_ps = ps2.tile([128, 128], f32, tag="g2", name="warm")
            for _ in range(24):
                nc.tensor.matmul(warm_ps[:], ident_sb[:], ident_sb[:],
                                 start=True, stop=True)

            # ---- state: c per half (fp32) ----
            c_half = [stp.tile([128, 128], f32, name=f"c{i}") for i in range(2)]
            for x in c_half:
                nc.vector.memset(x[:], 0.0)

            enc_t, feats_t, xgA_t, xgB_t = {}, {}, {}, {}
            houtA_t, houtB_t, hstepA_t, hstepB_t = {}, {}, {}, {}

            def load_enc(cc, eng=None):
                t_ = encp.tile([128, KDE * NCOL], b16, tag="enc", name="enc")
                (eng or nc.sync).dma_start(t_[:], encT_d[:, cc, :])
                enc_t[cc] = t_

            def g1_group(cc, m, prio=-60):
                # feats.T rows [128m, 128m+128) for chunk cc = feats k-chunk m
                if m == 0:
                    feats_t[cc] = [fp.tile([128, NCOL], b16, tag=f"feats{j}",
                                           name=f"feats{j}") for j in range(KDF)]
                ps = ps1.tile([128, NCOL], f32, tag="g1")
                e = enc_t[cc]
                for k in range(KDE):
                    nc.tensor.matmul(
                        ps[:],
                        wproj_sb[:, k * DF + m * 128: k * DF + m * 128 + 128],
                        e[:, k * NCOL:(k + 1) * NCOL],
                        start=(k == 0), stop=(k == KDE - 1),
                    )
                with tc.high_priority(prio):
                    nc.scalar.activation(feats_t[cc][m][:], ps[:], AF.Copy)

            def g2_group(cc, m, prio=-60):
                # XG.T rows [128m, 128m+128) for chunk cc (+ bias fold-in)
                if m == 0:
                    xgA_t[cc] = xgp.tile([128, 8 * NCOL], b16, tag="xgA", name="xgA")
                    xgB_t[cc] = xgp.tile([128, 8 * NCOL], b16, tag="xgB", name="xgB")
                ps = ps2.tile([128, NCOL], f32, tag="g2")
                f_ = feats_t[cc]
                q, mq = m // 4, m % 4
                for k in range(KDF):
                    nc.tensor.matmul(
                        ps[:],
                        wx_q[q][:, k * 512 + mq * 128: k * 512 + mq * 128 + 128],
                        f_[k][:],
                        start=(k == 0), stop=(k == KDF - 1),
                    )
                bias = biasr_sb[:, m * BL:(m + 1) * BL]
                bias3 = bias.rearrange("p (one b) -> p one b", one=1)
                xg = xgA_t[cc] if m < 8 else xgB_t[cc]
                with tc.high_priority(prio):
                    nc.vector.scalar_tensor_tensor(
                        xg[:, (m % 8) * NCOL:(m % 8 + 1) * NCOL],
                        ps[:], 1.0,
                        bias3.broadcast_to([128, CH, BL]),
                        op0=ALU.mult, op1=ALU.add,
                    )

            def rec_bank(t, gps, bank):
                # inject XG (clears the bank), then accumulate W_hh.T @ h.T
                cc, ts = t // CH, t % CH
                xg = (xgA_t if bank == 0 else xgB_t)[cc]
                xg3 = xg[:].rearrange("p (m n) -> p m n", m=8)
                nc.tensor.matmul(
                    gps[:],
                    ident_sb[:],
                    xg3[:, :, ts * BL:(ts + 1) * BL],
                    start=True, stop=(t == 0), skip_group_check=True,
                )
                if t == 0:
                    return  # h is zero at t=0: gates = XG only
                ccp, tsp = (t - 1) // CH, (t - 1) % CH
                if ccp == NCH - 1:
                    ha, hb = hstepA_t[t - 1], hstepB_t[t - 1]
                    tsp = 0
                else:
                    ha, hb = houtA_t[ccp], houtB_t[ccp]
                hsrc = (ha, ha, hb, hb)
                for k in range(KH):
                    rhs = hsrc[k][:, (k % 2) * CH * BL + tsp * BL:
                                  (k % 2) * CH * BL + tsp * BL + BL]
                    for m in range(bank * 8, bank * 8 + 8):
                        nc.tensor.matmul(
                            gps[:, (m % 8) * BL:(m % 8 + 1) * BL],
                            whh_k[k][:, m * 128:(m + 1) * 128],
                            rhs,
                            start=False,
                            stop=(k == KH - 1 and m % 8 == 7),
                            skip_group_check=True,
                        )

            def eltwise_half(t, gps, hf):
                # gate layout per bank: [f 0:128 | i 128:256 | o 256:384 | g 384:512]
                cc, ts = t // CH, t % CH
                act = ewp.tile([128, 512], f32, tag="act", name="act")
                cs = c_half[hf]
                nc.scalar.activation(act[:, 0:128], gps[:, 0:128], AF.Sigmoid)
                nc.scalar.activation(act[:, 384:512], gps[:, 384:512], AF.Tanh)
                nc.scalar.activation(act[:, 128:384], gps[:, 128:384], AF.Sigmoid)
                t2 = ewp.tile([128, 128], f32, tag="t2", name="t2")
                nc.gpsimd.tensor_mul(t2[:], act[:, 0:128], cs[:])
                t1 = ewp.tile([128, 128], f32, tag="t1", name="t1")
                nc.vector.tensor_mul(t1[:], act[:, 128:256], act[:, 384:512])
                nc.vector.tensor_add(cs[:], t1[:], t2[:])
                tc_ = ewp.tile([128, 128], f32, tag="tc", name="tc")
                nc.scalar.activation(tc_[:], cs[:], AF.Tanh)
                # h (state == output staging): bf16, scaled 1/WHH_SCALE
                if cc == NCH - 1:
                    ho = (hstepA_t if hf == 0 else hstepB_t)[t]
                    ts = 0          # per-step tile: always slot 0
                else:
                    ho = (houtA_t if hf == 0 else houtB_t)[cc]
                ho4 = ho[:].rearrange("p (k s b) -> p k s b", k=2, s=CH)
                so2 = act[:, 256:384].rearrange("p (k b) -> p k b", k=2)
                tc2 = tc_[:].rearrange("p (k b) -> p k b", k=2)
                nc.vector.scalar_tensor_tensor(
                    ho4[:, :, ts, :], so2, 1.0 / WHH_SCALE, tc2,
                    op0=ALU.mult, op1=ALU.mult,
                )

            # ---- prologue: the three transfers on g1/g2's critical path
            # (wproj above on sync, enc0, wx_q0) ride separate DMA queues so
            # they run concurrently; the rest stays serialized on sync
            load_enc(0, eng=nc.gpsimd)
            nc.scalar.dma_start(wx_q[0][:], wx_d[:, :, 0:512])
            nc.sync.dma_start(biasr_sb[:], biasr_d[:])
            for q in range(1, 4):
                nc.sync.dma_start(wx_q[q][:], wx_d[:, :, q * 512:(q + 1) * 512])
            load_enc(1)
            for k in range(KH):
                nc.sync.dma_start(whh_k[k][:], whh_d[:, k, :])
            for m in range(KDF):
                g1_group(0, m, prio=0)
            for m in range(MT):
                g2_group(0, m, prio=0)
            for m in range(KDF):
                g1_group(1, m, prio=0)

            # ---- main loop ----
            for t in range(T):
                cc, ts = t // CH, t % CH
                if cc == NCH - 1:
                    # final chunk: per-step tiles (slot 0 only) so each step's
                    # output DMAs immediately with no tile-level WAR hazard
                    hstepA_t[t] = hopA.tile([128, 2 * CH * BL], b16,
                                            tag="houtA", name="houtA")
                    hstepB_t[t] = hopB.tile([128, 2 * CH * BL], b16,
                                            tag="houtB", name="houtB")
                elif ts == 0:
                    houtA_t[cc] = hopA.tile([128, 2 * CH * BL], b16,
                                            tag="houtA", name="houtA")
                    houtB_t[cc] = hopB.tile([128, 2 * CH * BL], b16,
                                            tag="houtB", name="houtB")
                # separate psum tiles per half so eltwise(0) doesn't wait bank 1
                gpsA = psg.tile([128, 512], f32, tag="gatesA", name="gatesA")
                gpsB = psg.tile([128, 512], f32, tag="gatesB", name="gatesB")
                rec_bank(t, gpsA, 0)
                rec_bank(t, gpsB, 1)
                eltwise_half(t, gpsA, 0)
                eltwise_half(t, gpsB, 1)
                # GEMM quota for future chunks fills PE while eltwise runs
                if cc + 2 < NCH:
                    if ts == 0:
                        load_enc(cc + 2)
                    g1_group(cc + 2, ts)
                if cc + 1 < NCH:
                    for q in range(4):
                        g2_group(cc + 1, 4 * ts + q)
                if cc >= NCH - 2:
                    # no GEMM quota left here: idle-PE warmers hold HAM at
                    # full clock (they run while the eltwise chain is the
                    # bottleneck, so they cost no critical-path time)
                    wps = ps1.tile([128, NCOL], f32, tag="g1", name="wtail")
                    for _ in range(6):
                        nc.tensor.matmul(wps[:], ident_sb[:],
                                         wproj_sb[:, 0:NCOL],
                                         start=True, stop=True,
                                         skip_group_check=True)
                if cc == NCH - 1:
                    sA = hstepA_t[t][:].rearrange(
                        "p (k s b) -> p k s b", k=2, s=CH)
                    sB = hstepB_t[t][:].rearrange(
                        "p (k s b) -> p k s b", k=2, s=CH)
                    nc.sync.dma_start(hT_d[:, 0:2, t, :], sA[:, :, 0, :])
                    nc.sync.dma_start(hT_d[:, 2:4, t, :], sB[:, :, 0, :])
                elif ts == CH - 1:
                    ho4A = houtA_t[cc][:].rearrange(
                        "p (k s b) -> p k s b", k=2, s=CH)
                    ho4B = houtB_t[cc][:].rearrange(
                        "p (k s b) -> p k s b", k=2, s=CH)
                    nc.sync.dma_start(
                        hT_d[:, 0:2, cc * CH:(cc + 1) * CH, :], ho4A[:])
                    nc.sync.dma_start(
                        hT_d[:, 2:4, cc * CH:(cc + 1) * CH, :], ho4B[:])

    nc.compile()
    return nc


def _get_compiled():
    global _COMPILED
    if _COMPILED is None:
        _COMPILED = _build()
    return _COMPILED


def _prep_maps(outputs_encoder, start_ids, W_proj, b_proj, embed_table,
               W_ih, W_hh, b_ih, b_hh):
    outputs_encoder = np.asarray(outputs_encoder, np.float32)
    start_ids = np.asarray(start_ids)
    W_proj = np.asarray(W_proj, np.float32)
    b_proj = np.asarray(b_proj, np.float32)
    embed_table = np.asarray(embed_table, np.float32)
    W_ih = np.asarray(W_ih, np.float32)
    W_hh = np.asarray(W_hh, np.float32)
    b_ih = np.asarray(b_ih, np.float32)
    b_hh = np.asarray(b_hh, np.float32)

    # gate-row permutation: [f0 i0 o0 g0 f1 i1 o1 g1] (torch order i,f,g,o)
    perm = []
    for half in range(2):
        for g0 in (1, 0, 3, 2):
            base = g0 * H + half * 256
            perm.extend(range(base, base + 256))
    perm = np.asarray(perm)

    W_ih_p = W_ih[perm]
    W_hh_p = W_hh[perm]
    bvec_p = (b_ih + b_hh)[perm]
    W_x = W_ih_p[:, :DF]
    W_w = W_ih_p[:, DF:]

    word = embed_table[start_ids]                       # [B, DW]
    # full (t,b)-constant gate bias: word part + b_ih + b_hh + b_proj @ W_x.T
    biasw = word @ W_w.T + bvec_p[None, :] + (b_proj @ W_x.T)[None, :]  # [B, G4]

    del bvec_p
    wproj_arr = np.ascontiguousarray(
        W_proj.reshape(KDE, 128, DF).transpose(1, 0, 2)).astype(BF16)
    wx_arr = np.ascontiguousarray(
        W_x.T.reshape(KDF, 128, G4).transpose(1, 0, 2)).astype(BF16)
    whh_arr = np.ascontiguousarray(
        np.clip(W_hh_p.T * WHH_SCALE, -15.5, 15.5)
        .reshape(KH, 128, G4).transpose(1, 0, 2)).astype(F8E3)
    in_maps = []
    for c in range(NCORES):
        bsl = slice(c * BL, (c + 1) * BL)
        enc_c = outputs_encoder[bsl]                    # [64, 80, 1024]
        encT = np.ascontiguousarray(
            enc_c.transpose(2, 1, 0)                    # [1024, 80, 64]
            .reshape(KDE, 128, NCH, NCOL)
            .transpose(1, 2, 0, 3)                      # [128, NCH, KDE, NCOL]
            .reshape(128, NCH, KDE * NCOL)).astype(BF16)
        # gate-major bias [2048, 64] -> [128, m(16), 64]
        bgm = biasw[bsl].T.reshape(MT, 128, BL).transpose(1, 0, 2)  # [128,16,64]
        in_maps.append({
            "ident": np.eye(128, dtype=np.float32).astype(BF16),
            "encT": encT,
            "wproj": wproj_arr,
            "wx": wx_arr,
            "whh": whh_arr,
            "biasr": np.ascontiguousarray(bgm).astype(BF16),
        })
    return in_maps


def run_on_device(in_maps, trace=False):
    from concourse.bass_utils import run_bass_kernel_spmd
    nc = _get_compiled()
    return run_bass_kernel_spmd(
        nc, in_maps, core_ids=list(range(NCORES)), trace=trace)


def _unshard(res):
    out = np.empty((B, T, H), np.float32)
    for c in range(NCORES):
        hT = np.asarray(res.results[c]["hT"], dtype=np.float32)  # [128,4,80,64]
        out[c * BL:(c + 1) * BL] = (
            hT.transpose(3, 2, 1, 0).reshape(BL, T, H)) * WHH_SCALE
    return out


def kernel(**inputs):
    in_maps = _prep_maps(**inputs)
    res = None
    for attempt, pause in enumerate((0.0, 5.0, 60.0)):
        try:
            if pause:
                import time
                time.sleep(pause)
            res = run_on_device(in_maps)
            break
        except Exception:
            # the axon-proxied device occasionally reports a transient
            # NRT error or a wedged core; retries after a pause succeed
            if attempt == 2:
                raise
    return _unshard(res)
